# revision 12
# baseline (speedup 1.0000x reference)
"""GAT (2-layer, PyG-style) on 8 Trainium2 NeuronCores.

Strategy (dst-owner sharding, gather-free):
  - Nodes partitioned across 8 cores by dst id; every explicit edge plus one
    self-loop per node becomes a slot in a padded-CSR layout (128 dst rows
    per block, block slot-count L_b unified across cores for SPMD).
  - K1 (per core): transform own nodes h|a_s|a_d = x @ [W1*bn | As | Ad]
    -> htab shard (bf16, block-permuted order).
  - Host: concat shards, materialize the per-slot edge payload (h|a_s of the
    src node of every slot) in a partition-blocked sequential layout, so the
    edge kernels need no dma_gather (the Q7 descriptor-generation bottleneck
    of gather-based variants) — every DMA is a plain contiguous HWDGE read.
  - K2 (per core): per dst-block: sequential DMA of slot payloads,
    segment-softmax attention (denominator folded at the end), PSUM identity-
    matmul scatter, fused BN+ELU, layer-2 input transform -> h2|a_s2|a_d2.
  - Host: assemble + materialize layer-2 per-slot payload (f32).
  - K3 (per core): same edge stage with H=1, log_softmax with the ln() batched
    over all blocks at the end (avoids per-block activation-table reloads).
  - Host: un-permute rows, concat cores.
"""
import sys
import types

sys.path.insert(0, "/opt/trn_rl_repo")

import numpy as np
import ml_dtypes

BF16 = ml_dtypes.bfloat16

import concourse.bacc as bacc
import concourse.bass as bass
import concourse.mybir as mybir
from concourse.tile import TileContext
from concourse import bass_utils

F32 = mybir.dt.float32
BF = mybir.dt.bfloat16

NEG_SLOPE = 0.2
BN_EPS = 1e-5

W1CH = 136          # per-slot layer-1 payload elems (h 128 | a_s 8), bf16
W2CH = 42           # per-slot layer-2 payload elems (h2 40 | a_s2 | a_d2), f32
KOUT = 144          # K1 output row (h 128 | a_s 8 | a_d 8)


# ---------------------------------------------------------------- config
def make_cfg(N=50000, E=800000, Fin=128, H=8, C1=16, Fout=40, ncores=8):
    cfg = {}
    cfg["N"], cfg["E"] = N, E
    cfg["Fin"], cfg["H"], cfg["C1"], cfg["Fout"] = Fin, H, C1, Fout
    cfg["HC"] = H * C1
    cfg["ncores"] = ncores
    assert N % ncores == 0
    cfg["npc"] = N // ncores                       # nodes per core
    cfg["nblk"] = (cfg["npc"] + 127) // 128        # dst blocks per core
    cfg["nrows"] = cfg["nblk"] * 128               # shard rows (padded)
    assert Fin == 128 and cfg["HC"] == 128
    return cfg


# ------------------------------------------------------------ host graph prep
def preprocess_graph(cfg, edge_index):
    """Per-core padded-CSR slot structure (self-loops included as slots)."""
    N, ncores, npc = cfg["N"], cfg["ncores"], cfg["npc"]
    nblk, nrows = cfg["nblk"], cfg["nrows"]
    src = np.asarray(edge_index[0], np.int64)
    dst = np.asarray(edge_index[1], np.int64)

    cores = []
    for k in range(ncores):
        m = (dst // npc) == k
        own = np.arange(npc, dtype=np.int64)
        s_k = np.concatenate([src[m], own + k * npc])     # + self-loops
        d_loc = np.concatenate([dst[m] - k * npc, own])
        deg = np.bincount(d_loc, minlength=npc)
        order = np.argsort(-deg, kind="stable")
        row2node = np.full(nrows, -1, np.int64)
        row2node[:npc] = order + k * npc
        fin_rank = np.full(npc, -1, np.int64)
        fin_rank[order] = np.arange(npc)
        degs = deg[order]
        L = np.zeros(nblk, np.int64)
        for b in range(nblk):
            sl = slice(b * 128, min((b + 1) * 128, npc))
            L[b] = max(1, int(degs[sl].max())) if sl.start < npc else 1
        cores.append(dict(s_k=s_k, d_loc=d_loc, row2node=row2node,
                          fin_rank=fin_rank, L=L))

    # unify per-block slot counts across cores (blocks already deg-sorted)
    Lu = np.zeros(nblk, np.int64)
    for c in cores:
        Lu = np.maximum(Lu, c["L"])
    offs = np.zeros(nblk + 1, np.int64)
    offs[1:] = np.cumsum(Lu)
    total_cols = int(offs[-1])

    # slot_src[b]: [Lu[b], 128] global src node id, -1 = pad
    for c in cores:
        re = c["fin_rank"][c["d_loc"]]
        okey = np.argsort(re, kind="stable")
        rr = re[okey]
        ss = c["s_k"][okey]
        jj = np.arange(len(rr)) - np.searchsorted(rr, rr, side="left")
        slot_src = [np.full((int(Lu[b]), 128), -1, np.int64) for b in range(nblk)]
        b_e = rr // 128
        p_e = rr % 128
        for b in range(nblk):
            sel = b_e == b
            slot_src[b][jj[sel], p_e[sel]] = ss[sel]
        c["slot_src"] = slot_src

    return dict(cores=cores, Lu=Lu, offs=offs, total_cols=total_cols)


def materialize_slots(cfg, g, tab_ext, W):
    """tab_ext: [N+1, W] payload per node (+ sentinel row N).
    Returns per-core [128, total_cols*W] partition-blocked slot payload."""
    nblk = cfg["nblk"]
    N = cfg["N"]
    out = []
    for c in g["cores"]:
        parts = []
        for b in range(nblk):
            sl = c["slot_src"][b]                      # [L, 128]
            idx = np.where(sl >= 0, sl, N)
            pay = tab_ext[idx]                         # [L, 128, W]
            parts.append(np.ascontiguousarray(pay.transpose(1, 0, 2))
                         .reshape(128, -1))
        out.append(np.concatenate(parts, axis=1))
    return out


# ------------------------------------------------------------ host param prep
def preprocess_params(cfg, W1, att_src1, att_dst1, b1, bn_gamma, bn_beta,
                      bn_mean, bn_var, W2, att_src2, att_dst2, b2):
    H, C1v, HC, Fout = cfg["H"], cfg["C1"], cfg["HC"], cfg["Fout"]
    W1 = W1.astype(np.float64)
    W2 = W2.astype(np.float64)
    a_feat = bn_gamma.astype(np.float64) / np.sqrt(bn_var.astype(np.float64) + BN_EPS)
    b_feat = (b1.astype(np.float64) - bn_mean.astype(np.float64)) * a_feat \
        + bn_beta.astype(np.float64)
    As = np.zeros((HC, H))
    Ad = np.zeros((HC, H))
    for h in range(H):
        As[h * C1v:(h + 1) * C1v, h] = att_src1[h].astype(np.float64)
        Ad[h * C1v:(h + 1) * C1v, h] = att_dst1[h].astype(np.float64)
    As_eff = W1 @ As
    Ad_eff = W1 @ Ad
    colmap = np.array([h * C1v + c for c in range(C1v) for h in range(H)])
    W1a_r = (W1 * a_feat[None, :])[:, colmap]
    W1cat2 = np.concatenate([W1a_r, As_eff, Ad_eff], axis=1)  # [Fin, 152]
    b_b = b_feat[colmap]
    w_s2 = W2 @ att_src2[0].astype(np.float64)
    w_d2 = W2 @ att_dst2[0].astype(np.float64)
    W2cat = np.concatenate([W2, w_s2[:, None], w_d2[:, None]], axis=1)[colmap, :]
    c2 = W2cat.sum(axis=0)                                    # [Fout+2]
    return dict(
        W1cat2=W1cat2.astype(np.float32).astype(BF16),
        b_bcast=np.broadcast_to(b_b.astype(np.float32).astype(BF16), (128, HC)).copy(),
        W2cat=W2cat.astype(np.float32).astype(BF16),
        c2b=np.broadcast_to(c2.astype(np.float32), (128, Fout + 2)).copy(),
        b2c=np.broadcast_to(b2.astype(np.float32), (128, Fout)).copy(),
        identb=np.eye(128, dtype=np.float32).astype(BF16),
    )


# ---------------------------------------------------------------- kernel 1
def build_kernel_1(cfg):
    """Own-node transform: htab[r] = xtp[:,r]^T @ W1cat2."""
    nblk, nrows = cfg["nblk"], cfg["nrows"]
    nc = bacc.Bacc("TRN2", target_bir_lowering=False, debug=False)
    xtp_d = nc.dram_tensor("xTP", [128, nrows], BF, kind="ExternalInput")
    w1_d = nc.dram_tensor("W1cat2", [128, KOUT], BF, kind="ExternalInput")
    htab = nc.dram_tensor("htab", [nrows, KOUT], BF, kind="ExternalOutput")

    with TileContext(nc) as tc:
        with tc.tile_pool(name="consts", bufs=1) as cp:
            xtp = cp.tile([128, nrows], BF)
            nc.sync.dma_start(out=xtp[:], in_=xtp_d[:])
            w1c = cp.tile([128, KOUT], BF)
            nc.sync.dma_start(out=w1c[:], in_=w1_d[:])
            with tc.tile_pool(name="t", bufs=4) as ap, \
                 tc.tile_pool(name="ps", bufs=4, space="PSUM") as aps:
                MB = 8
                for b0 in range(0, nblk, MB):
                    nb = min(MB, nblk - b0)
                    st = ap.tile([128, MB * KOUT], BF, tag="st")
                    for bi in range(nb):
                        b = b0 + bi
                        ps = aps.tile([128, KOUT], F32, tag="ps")
                        nc.tensor.matmul(ps[:], lhsT=xtp[:, b * 128:(b + 1) * 128],
                                         rhs=w1c[:], start=True, stop=True)
                        if bi % 2 == 0:
                            nc.vector.tensor_copy(
                                out=st[:, bi * KOUT:(bi + 1) * KOUT], in_=ps[:])
                        else:
                            nc.scalar.copy(
                                out=st[:, bi * KOUT:(bi + 1) * KOUT], in_=ps[:])
                    dv = htab[b0 * 128:(b0 + nb) * 128, :] \
                        .rearrange("(b p) c -> p b c", p=128)
                    sv = st[:, 0:nb * KOUT].rearrange("p (b c) -> p b c", c=KOUT)
                    nc.sync.dma_start(out=dv, in_=sv)
    nc.finalize()
    return nc


# ---------------------------------------------------------------- kernel 2
def build_kernel_2(cfg, g):
    """Layer-1 edge stage on host-materialized slot payloads (no gathers)."""
    HC, H, Fout = cfg["HC"], cfg["H"], cfg["Fout"]
    nblk, nrows = cfg["nblk"], cfg["nrows"]
    Lu, offs, total_cols = g["Lu"], g["offs"], g["total_cols"]

    nc = bacc.Bacc("TRN2", target_bir_lowering=False, debug=False)
    hg_d = nc.dram_tensor("hg", [128, total_cols * W1CH], BF, kind="ExternalInput")
    ad_d = nc.dram_tensor("adall", [128, nblk * H], BF, kind="ExternalInput")
    bb_d = nc.dram_tensor("b_bcast", [128, HC], BF, kind="ExternalInput")
    w2_d = nc.dram_tensor("W2cat", [128, Fout + 2], BF, kind="ExternalInput")
    c2_d = nc.dram_tensor("c2b", [128, Fout + 2], F32, kind="ExternalInput")
    id_d = nc.dram_tensor("identb", [128, 128], BF, kind="ExternalInput")
    shard = nc.dram_tensor("shard", [nrows, Fout + 2], F32, kind="ExternalOutput")
    Lmax = int(Lu.max())

    with TileContext(nc) as tc:
        with tc.tile_pool(name="consts", bufs=1) as cp:
            adall = cp.tile([128, nblk * H], BF)
            nc.sync.dma_start(out=adall[:], in_=ad_d[:])
            bb = cp.tile([128, HC], BF)
            nc.sync.dma_start(out=bb[:], in_=bb_d[:])
            w2c = cp.tile([128, Fout + 2], BF)
            nc.sync.dma_start(out=w2c[:], in_=w2_d[:])
            c2b = cp.tile([128, Fout + 2], F32)
            nc.sync.dma_start(out=c2b[:], in_=c2_d[:])
            idb = cp.tile([128, 128], BF)
            nc.sync.dma_start(out=idb[:], in_=id_d[:])

            with tc.tile_pool(name="e2", bufs=4) as ep, \
                 tc.tile_pool(name="e2g", bufs=3) as gp, \
                 tc.tile_pool(name="e2m", bufs=3) as mp, \
                 tc.tile_pool(name="e2ps", bufs=3, space="PSUM") as eps, \
                 tc.tile_pool(name="e2ps2", bufs=2, space="PSUM") as eps2:
                # software pipeline: pre+scatter for block i; PSUM-dependent
                # epilogue for block i-1; transpose/W2 for block i-2 — so no
                # in-order engine queue ever waits on an unfinished producer.
                c1 = None
                c2 = None
                for i in range(nblk + 2):
                    if i < nblk:
                        b = i
                        lt = int(Lu[b])
                        off = int(offs[b])
                        gt = gp.tile([128, Lmax * W1CH], BF, tag="g")
                        nc.sync.dma_start(
                            out=gt[:, 0:lt * W1CH],
                            in_=hg_d[:, off * W1CH:(off + lt) * W1CH])
                        gv = gt[:, 0:lt * W1CH].rearrange("p (l w) -> p l w", w=W1CH)
                        adb = adall[:, b * H:(b + 1) * H] \
                            .unsqueeze(1).to_broadcast([128, lt, H])
                        # e = a_s + a_d ; p = exp(leaky_relu(e))
                        e = ep.tile([128, lt * H], BF, tag="e")
                        nc.vector.tensor_tensor(
                            out=e[:].rearrange("p (l h) -> p l h", h=H),
                            in0=gv[:, :, HC:W1CH], in1=adb, op=mybir.AluOpType.add)
                        lr = ep.tile([128, lt * H], BF, tag="lr")
                        nc.scalar.activation(out=lr[:], in_=e[:],
                                             func=mybir.ActivationFunctionType.Lrelu,
                                             alpha=NEG_SLOPE)
                        p = ep.tile([128, lt * H], BF, tag="p")
                        nc.scalar.activation(out=p[:], in_=lr[:],
                                             func=mybir.ActivationFunctionType.Exp)
                    else:
                        b = None
                    # ---- epilogue for block i-1 (fills the p-wait gap)
                    if c1 is not None:
                        pso1, rden1, zzt = c1["pso"], c1["rden"], c1["zz"]
                        v0 = ep.tile([128, HC], BF, tag="v0")
                        rexp = rden1[:].unsqueeze(1).to_broadcast([128, HC // H, H])
                        nc.vector.tensor_tensor(
                            out=v0[:].rearrange("p (c h) -> p c h", h=H),
                            in0=pso1[:].rearrange("p (c h) -> p c h", h=H),
                            in1=rexp, op=mybir.AluOpType.mult)
                        v = ep.tile([128, HC], BF, tag="v")
                        nc.gpsimd.tensor_add(out=v[:], in0=v0[:], in1=bb[:])
                        rr = ep.tile([128, HC], BF, tag="rr")
                        nc.scalar.activation(out=rr[:], in_=v[:],
                                             func=mybir.ActivationFunctionType.Relu)
                        mn = ep.tile([128, HC], BF, tag="mn")
                        nc.gpsimd.tensor_tensor(out=mn[:], in0=v[:], in1=rr[:],
                                                op=mybir.AluOpType.subtract)
                        u = ep.tile([128, HC], BF, tag="u")
                        nc.scalar.activation(out=u[:], in_=mn[:],
                                             func=mybir.ActivationFunctionType.Exp)
                        nc.gpsimd.tensor_add(out=zzt[:], in0=rr[:], in1=u[:])
                    # ---- layer-2 transform for block i-2
                    if c2 is not None:
                        pso2, zz2, b2i = c2["pso"], c2["zz"], c2["b"]
                        pst = eps2.tile([128, 128], BF, tag="pst")
                        nc.tensor.transpose(out=pst[:], in_=zz2[:], identity=idb[:])
                        zt = ep.tile([128, 128], BF, tag="zt")
                        nc.scalar.copy(out=zt[:], in_=pst[:])
                        ph = eps2.tile([128, Fout + 2], F32, tag="ph")
                        nc.tensor.matmul(ph[:], lhsT=zt[:], rhs=w2c[:],
                                         start=True, stop=True)
                        h2a = ep.tile([128, Fout + 2], F32, tag="h2a")
                        nc.vector.tensor_tensor(out=h2a[:], in0=ph[:], in1=c2b[:],
                                                op=mybir.AluOpType.subtract)
                        nc.sync.dma_start(out=shard[b2i * 128:(b2i + 1) * 128, :],
                                          in_=h2a[:])
                    # ---- rest of pre + scatter for block i
                    if i < nblk:
                        den = ep.tile([128, H], F32, tag="den")
                        nc.vector.tensor_reduce(
                            out=den[:], in_=p[:].rearrange("p (l h) -> p h l", h=H),
                            axis=mybir.AxisListType.X, op=mybir.AluOpType.add)
                        rden = ep.tile([128, H], F32, tag="rden")
                        nc.vector.reciprocal(out=rden[:], in_=den[:])
                        m = mp.tile([128, Lmax * HC], BF, tag="m")
                        hview = gv[:, :, 0:HC].rearrange("p l (c h) -> p l c h", h=H)
                        pexp = p[:].rearrange("p (l h) -> p l h", h=H) \
                            .unsqueeze(2).to_broadcast([128, lt, HC // H, H])
                        nc.vector.tensor_tensor(
                            out=m[:, 0:lt * HC].rearrange(
                                "p (l c h) -> p l c h", c=HC // H, h=H),
                            in0=hview, in1=pexp, op=mybir.AluOpType.mult)
                        pso = eps.tile([128, HC], F32, tag="pso")
                        for j in range(lt):
                            nc.tensor.matmul(pso[:], lhsT=idb[:],
                                             rhs=m[:, j * HC:(j + 1) * HC],
                                             start=(j == 0), stop=(j == lt - 1))
                        zzn = ep.tile([128, HC], BF, tag="zz")
                        nxt = {"pso": pso, "rden": rden, "zz": zzn, "b": b}
                    else:
                        nxt = None
                    c2 = c1
                    c1 = nxt
    nc.finalize()
    return nc


# ---------------------------------------------------------------- kernel 3
def build_kernel_3(cfg, g):
    """Layer-2 edge stage (H=1) + log_softmax with batched ln()."""
    Fout = cfg["Fout"]
    nblk = cfg["nblk"]
    Lu, offs, total_cols = g["Lu"], g["offs"], g["total_cols"]

    nc = bacc.Bacc("TRN2", target_bir_lowering=False, debug=False)
    hg_d = nc.dram_tensor("hg2", [128, total_cols * W2CH], F32, kind="ExternalInput")
    ad_d = nc.dram_tensor("ad2all", [128, nblk], F32, kind="ExternalInput")
    b2_d = nc.dram_tensor("b2c", [128, Fout], F32, kind="ExternalInput")
    id_d = nc.dram_tensor("identb", [128, 128], BF, kind="ExternalInput")
    outsh = nc.dram_tensor("outsh", [128, nblk * Fout], F32, kind="ExternalOutput")
    Lmax = int(Lu.max())

    with TileContext(nc) as tc:
        with tc.tile_pool(name="consts", bufs=1) as cp:
            ad2 = cp.tile([128, nblk], F32)
            nc.sync.dma_start(out=ad2[:], in_=ad_d[:])
            b2c = cp.tile([128, Fout], F32)
            nc.sync.dma_start(out=b2c[:], in_=b2_d[:])
            idb = cp.tile([128, 128], BF)
            nc.sync.dma_start(out=idb[:], in_=id_d[:])
            obuf = cp.tile([128, nblk * Fout], F32)
            sebuf = cp.tile([128, nblk], F32)

            with tc.tile_pool(name="e3", bufs=6) as ep, \
                 tc.tile_pool(name="e3g", bufs=3) as gp, \
                 tc.tile_pool(name="e3m", bufs=3) as mp, \
                 tc.tile_pool(name="e3ps", bufs=3, space="PSUM") as eps:
                c1 = None
                for i in range(nblk + 1):
                    if i < nblk:
                        b = i
                        lt = int(Lu[b])
                        off = int(offs[b])
                        gt = gp.tile([128, Lmax * W2CH], F32, tag="g")
                        nc.sync.dma_start(
                            out=gt[:, 0:lt * W2CH],
                            in_=hg_d[:, off * W2CH:(off + lt) * W2CH])
                        gv = gt[:, 0:lt * W2CH].rearrange("p (l w) -> p l w", w=W2CH)
                        adb = ad2[:, b:b + 1].to_broadcast([128, lt])
                        e2 = ep.tile([128, lt], F32, tag="e2")
                        nc.vector.tensor_tensor(out=e2[:],
                                                in0=gv[:, :, Fout:Fout + 1].squeeze(),
                                                in1=adb, op=mybir.AluOpType.add)
                        lr2 = ep.tile([128, lt], F32, tag="lr2")
                        nc.scalar.activation(out=lr2[:], in_=e2[:],
                                             func=mybir.ActivationFunctionType.Lrelu,
                                             alpha=NEG_SLOPE)
                        # p2 = exp(leaky(e2)); den2 comes free via accum (H=1)
                        p2 = ep.tile([128, lt], F32, tag="p2")
                        den2 = ep.tile([128, 1], F32, tag="den2")
                        nc.scalar.activation(out=p2[:], in_=lr2[:],
                                             func=mybir.ActivationFunctionType.Exp,
                                             accum_out=den2[:])
                    # ---- epilogue for block i-1 (fills the p2-wait gap)
                    if c1 is not None:
                        ps21, rden21, b1i = c1["ps2"], c1["rden2"], c1["b"]
                        o2 = ep.tile([128, Fout], F32, tag="o2")
                        r2e = rden21[:].to_broadcast([128, Fout])
                        nc.vector.tensor_tensor(out=o2[:], in0=ps21[:], in1=r2e,
                                                op=mybir.AluOpType.mult)
                        # o3 written straight into the persistent output buffer;
                        # log-softmax runs without a max-shift (f32-safe range)
                        ob = obuf[:, b1i * Fout:(b1i + 1) * Fout]
                        nc.vector.tensor_tensor(out=ob, in0=o2[:], in1=b2c[:],
                                                op=mybir.AluOpType.add)
                        ex = ep.tile([128, Fout], F32, tag="ex")
                        nc.scalar.activation(out=ex[:], in_=ob,
                                             func=mybir.ActivationFunctionType.Exp,
                                             accum_out=sebuf[:, b1i:b1i + 1])
                    if i < nblk:
                        rden2 = ep.tile([128, 1], F32, tag="rden2")
                        nc.vector.reciprocal(out=rden2[:], in_=den2[:])
                        m2 = mp.tile([128, Lmax * Fout], BF, tag="m2")
                        p2e = p2[:].unsqueeze(2).to_broadcast([128, lt, Fout])
                        nc.vector.tensor_tensor(
                            out=m2[:, 0:lt * Fout].rearrange("p (l f) -> p l f", f=Fout),
                            in0=gv[:, :, 0:Fout], in1=p2e, op=mybir.AluOpType.mult)
                        ps2 = eps.tile([128, Fout], F32, tag="ps2")
                        for j in range(lt):
                            nc.tensor.matmul(ps2[:], lhsT=idb[:],
                                             rhs=m2[:, j * Fout:(j + 1) * Fout],
                                             start=(j == 0), stop=(j == lt - 1))
                        c1 = {"ps2": ps2, "rden2": rden2, "b": b}
                    else:
                        c1 = None
                # batched ln over all blocks, then one fused subtract + DMA out
                ls = cp.tile([128, nblk], F32)
                nc.scalar.activation(out=ls[:], in_=sebuf[:],
                                     func=mybir.ActivationFunctionType.Ln)
                ov = cp.tile([128, nblk * Fout], F32)
                lsv = ls[:].unsqueeze(2).to_broadcast([128, nblk, Fout])
                nc.vector.tensor_tensor(
                    out=ov[:].rearrange("p (b f) -> p b f", f=Fout),
                    in0=obuf[:].rearrange("p (b f) -> p b f", f=Fout),
                    in1=lsv, op=mybir.AluOpType.subtract)
                nc.sync.dma_start(out=outsh[:], in_=ov[:])
    nc.finalize()
    return nc


# ---------------------------------------------------------------- runner
_TRACE = False
last_times = {}


def _run_spmd(nc, in_maps, ncores):
    kw = {}
    if _TRACE:
        _install_hook()
        kw["trace"] = True
    return bass_utils.run_bass_kernel_spmd(nc, in_maps, core_ids=list(range(ncores)), **kw)


def _install_hook():
    try:
        import antenv
        if "antenv.axon_hooks" not in sys.modules:
            hooks_mod = types.ModuleType("antenv.axon_hooks")
            _h = [None]
            hooks_mod.set_axon_ntff_profile_hook = lambda h: _h.__setitem__(0, h)
            hooks_mod.get_axon_ntff_profile_hook = lambda: _h[0]
            sys.modules["antenv.axon_hooks"] = hooks_mod
            antenv.axon_hooks = hooks_mod
            from trn_agent_boot.trn_boot import _ntff_profile_via_ctypes
            hooks_mod.set_axon_ntff_profile_hook(
                _ntff_profile_via_ctypes('/opt/axon/libaxon_pjrt.so'))
    except Exception as e:  # pragma: no cover
        print("hook install failed:", e, file=sys.stderr)


def gat_forward(cfg, inputs):
    N, Fout, H, HC = cfg["N"], cfg["Fout"], cfg["H"], cfg["HC"]
    ncores, npc, nrows, nblk = cfg["ncores"], cfg["npc"], cfg["nrows"], cfg["nblk"]
    x = np.asarray(inputs["x"], np.float32)
    edge_index = np.asarray(inputs["edge_index"])

    g = preprocess_graph(cfg, edge_index)
    pp = preprocess_params(cfg, *[np.asarray(inputs[k]) for k in
                                  ("W1", "att_src1", "att_dst1", "b1", "bn_gamma",
                                   "bn_beta", "bn_mean", "bn_var", "W2",
                                   "att_src2", "att_dst2", "b2")])

    # ---- K1: per-core own-node transform
    nc1 = build_kernel_1(cfg)
    in1 = []
    for k in range(ncores):
        c = g["cores"][k]
        xtp = np.zeros((128, nrows), np.float32)
        valid = c["row2node"] >= 0
        xtp[:, valid] = x[c["row2node"][valid]].T
        in1.append({"xTP": xtp.astype(BF16), "W1cat2": pp["W1cat2"]})
    res1 = _run_spmd(nc1, in1, ncores)
    last_times["K1"] = res1.exec_time_ns

    # ---- host: assemble h|a_s table + per-core a_d, materialize slots
    htab_all = np.zeros((N + 1, KOUT), BF16)
    htab_all[N, HC:HC + H] = BF16(-1e30)           # sentinel: a_s = -inf
    for k in range(ncores):
        sh = res1.results[k]["htab"]
        c = g["cores"][k]
        valid = c["row2node"] >= 0
        htab_all[c["row2node"][valid]] = sh[valid]
    hg_cores = materialize_slots(cfg, g, htab_all[:, :W1CH], W1CH)

    nc2 = build_kernel_2(cfg, g)
    in2 = []
    for k in range(ncores):
        c = g["cores"][k]
        adall = np.zeros((128, nblk * H), BF16)
        r2n = c["row2node"].reshape(nblk, 128)
        for b in range(nblk):
            vb = r2n[b] >= 0
            adall[vb, b * H:(b + 1) * H] = htab_all[r2n[b][vb], HC + H:KOUT]
        in2.append({"hg": hg_cores[k], "adall": adall,
                    "b_bcast": pp["b_bcast"], "W2cat": pp["W2cat"],
                    "c2b": pp["c2b"], "identb": pp["identb"]})
    res2 = _run_spmd(nc2, in2, ncores)
    last_times["K2"] = res2.exec_time_ns

    # ---- host: assemble layer-2 table, materialize slots (f32)
    h2a_all = np.zeros((N + 1, W2CH), np.float32)
    h2a_all[N, Fout] = -1e30                       # sentinel: a_s2 = -inf
    for k in range(ncores):
        sh = res2.results[k]["shard"]
        c = g["cores"][k]
        valid = c["row2node"] >= 0
        h2a_all[c["row2node"][valid]] = sh[valid]
    hg2_cores = materialize_slots(cfg, g, h2a_all, W2CH)

    nc3 = build_kernel_3(cfg, g)
    in3 = []
    for k in range(ncores):
        c = g["cores"][k]
        ad2all = np.zeros((128, nblk), np.float32)
        r2n = c["row2node"].reshape(nblk, 128)
        for b in range(nblk):
            vb = r2n[b] >= 0
            ad2all[vb, b] = h2a_all[r2n[b][vb], Fout + 1]
        in3.append({"hg2": hg2_cores[k], "ad2all": ad2all,
                    "b2c": pp["b2c"], "identb": pp["identb"]})
    res3 = _run_spmd(nc3, in3, ncores)
    last_times["K3"] = res3.exec_time_ns

    out = np.zeros((N, Fout), np.float32)
    for k in range(ncores):
        sh = res3.results[k]["outsh"]              # [128, nblk*Fout]
        c = g["cores"][k]
        vals = sh.reshape(128, nblk, Fout).transpose(1, 0, 2).reshape(nrows, Fout)
        valid = c["row2node"] >= 0
        out[c["row2node"][valid]] = vals[valid]
    return out


def kernel(**inputs):
    cfg = make_cfg()
    return gat_forward(cfg, inputs)


# revision 13
# speedup vs baseline: 1.1999x; 1.1999x over previous
"""GAT (2-layer, PyG-style) on 8 Trainium2 NeuronCores.

Strategy (dst-owner sharding, gather-free):
  - Nodes partitioned across 8 cores by dst id; every explicit edge plus one
    self-loop per node becomes a slot in a padded-CSR layout (128 dst rows
    per block, block slot-count L_b unified across cores for SPMD).
  - K1 (per core): transform own nodes h|a_s|a_d = x @ [W1*bn | As | Ad]
    -> htab shard (bf16, block-permuted order).
  - Host: concat shards, materialize the per-slot edge payload (h|a_s of the
    src node of every slot) in a partition-blocked sequential layout, so the
    edge kernels need no dma_gather (the Q7 descriptor-generation bottleneck
    of gather-based variants) — every DMA is a plain contiguous HWDGE read.
  - K2 (per core): per dst-block: sequential DMA of slot payloads,
    segment-softmax attention (denominator folded at the end), PSUM identity-
    matmul scatter, fused BN+ELU, layer-2 input transform -> h2|a_s2|a_d2.
  - Host: assemble + materialize layer-2 per-slot payload (f32).
  - K3 (per core): same edge stage with H=1, log_softmax with the ln() batched
    over all blocks at the end (avoids per-block activation-table reloads).
  - Host: un-permute rows, concat cores.
"""
import sys
import types

sys.path.insert(0, "/opt/trn_rl_repo")

import numpy as np
import ml_dtypes

BF16 = ml_dtypes.bfloat16

import concourse.bacc as bacc
import concourse.bass as bass
import concourse.mybir as mybir
from concourse.tile import TileContext
from concourse import bass_utils

F32 = mybir.dt.float32
BF = mybir.dt.bfloat16

NEG_SLOPE = 0.2
BN_EPS = 1e-5

W1CH = 136          # per-slot layer-1 payload elems (h 128 | a_s 8), bf16
W2CH = 42           # per-slot layer-2 payload elems (h2 40 | a_s2 | a_d2), f32
KOUT = 144          # K1 output row (h 128 | a_s 8 | a_d 8)


# ---------------------------------------------------------------- config
def make_cfg(N=50000, E=800000, Fin=128, H=8, C1=16, Fout=40, ncores=8):
    cfg = {}
    cfg["N"], cfg["E"] = N, E
    cfg["Fin"], cfg["H"], cfg["C1"], cfg["Fout"] = Fin, H, C1, Fout
    cfg["HC"] = H * C1
    cfg["ncores"] = ncores
    assert N % ncores == 0
    cfg["npc"] = N // ncores                       # nodes per core
    cfg["nblk"] = (cfg["npc"] + 127) // 128        # dst blocks per core
    cfg["nrows"] = cfg["nblk"] * 128               # shard rows (padded)
    assert Fin == 128 and cfg["HC"] == 128
    return cfg


# ------------------------------------------------------------ host graph prep
def preprocess_graph(cfg, edge_index):
    """Per-core padded-CSR slot structure (self-loops included as slots)."""
    N, ncores, npc = cfg["N"], cfg["ncores"], cfg["npc"]
    nblk, nrows = cfg["nblk"], cfg["nrows"]
    src = np.asarray(edge_index[0], np.int64)
    dst = np.asarray(edge_index[1], np.int64)

    cores = []
    for k in range(ncores):
        m = (dst // npc) == k
        own = np.arange(npc, dtype=np.int64)
        s_k = np.concatenate([src[m], own + k * npc])     # + self-loops
        d_loc = np.concatenate([dst[m] - k * npc, own])
        deg = np.bincount(d_loc, minlength=npc)
        order = np.argsort(-deg, kind="stable")
        row2node = np.full(nrows, -1, np.int64)
        row2node[:npc] = order + k * npc
        fin_rank = np.full(npc, -1, np.int64)
        fin_rank[order] = np.arange(npc)
        degs = deg[order]
        L = np.zeros(nblk, np.int64)
        for b in range(nblk):
            sl = slice(b * 128, min((b + 1) * 128, npc))
            L[b] = max(1, int(degs[sl].max())) if sl.start < npc else 1
        cores.append(dict(s_k=s_k, d_loc=d_loc, row2node=row2node,
                          fin_rank=fin_rank, L=L))

    # unify per-block slot counts across cores (blocks already deg-sorted)
    Lu = np.zeros(nblk, np.int64)
    for c in cores:
        Lu = np.maximum(Lu, c["L"])
    offs = np.zeros(nblk + 1, np.int64)
    offs[1:] = np.cumsum(Lu)
    total_cols = int(offs[-1])

    # slot_src[b]: [Lu[b], 128] global src node id, -1 = pad
    for c in cores:
        re = c["fin_rank"][c["d_loc"]]
        okey = np.argsort(re, kind="stable")
        rr = re[okey]
        ss = c["s_k"][okey]
        jj = np.arange(len(rr)) - np.searchsorted(rr, rr, side="left")
        slot_src = [np.full((int(Lu[b]), 128), -1, np.int64) for b in range(nblk)]
        b_e = rr // 128
        p_e = rr % 128
        for b in range(nblk):
            sel = b_e == b
            slot_src[b][jj[sel], p_e[sel]] = ss[sel]
        c["slot_src"] = slot_src

    return dict(cores=cores, Lu=Lu, offs=offs, total_cols=total_cols)


def materialize_slots(cfg, g, tab_ext, W):
    """tab_ext: [N+1, W] payload per node (+ sentinel row N).
    Returns per-core [128, total_cols*W] partition-blocked slot payload."""
    nblk = cfg["nblk"]
    N = cfg["N"]
    out = []
    for c in g["cores"]:
        parts = []
        for b in range(nblk):
            sl = c["slot_src"][b]                      # [L, 128]
            idx = np.where(sl >= 0, sl, N)
            pay = tab_ext[idx]                         # [L, 128, W]
            parts.append(np.ascontiguousarray(pay.transpose(1, 0, 2))
                         .reshape(128, -1))
        out.append(np.concatenate(parts, axis=1))
    return out


# ------------------------------------------------------------ host param prep
def preprocess_params(cfg, W1, att_src1, att_dst1, b1, bn_gamma, bn_beta,
                      bn_mean, bn_var, W2, att_src2, att_dst2, b2):
    H, C1v, HC, Fout = cfg["H"], cfg["C1"], cfg["HC"], cfg["Fout"]
    W1 = W1.astype(np.float64)
    W2 = W2.astype(np.float64)
    a_feat = bn_gamma.astype(np.float64) / np.sqrt(bn_var.astype(np.float64) + BN_EPS)
    b_feat = (b1.astype(np.float64) - bn_mean.astype(np.float64)) * a_feat \
        + bn_beta.astype(np.float64)
    As = np.zeros((HC, H))
    Ad = np.zeros((HC, H))
    for h in range(H):
        As[h * C1v:(h + 1) * C1v, h] = att_src1[h].astype(np.float64)
        Ad[h * C1v:(h + 1) * C1v, h] = att_dst1[h].astype(np.float64)
    As_eff = W1 @ As
    Ad_eff = W1 @ Ad
    colmap = np.array([h * C1v + c for c in range(C1v) for h in range(H)])
    W1a_r = (W1 * a_feat[None, :])[:, colmap]
    W1cat2 = np.concatenate([W1a_r, As_eff, Ad_eff], axis=1)  # [Fin, 152]
    b_b = b_feat[colmap]
    w_s2 = W2 @ att_src2[0].astype(np.float64)
    w_d2 = W2 @ att_dst2[0].astype(np.float64)
    W2cat = np.concatenate([W2, w_s2[:, None], w_d2[:, None]], axis=1)[colmap, :]
    c2 = W2cat.sum(axis=0)                                    # [Fout+2]
    return dict(
        W1cat2=W1cat2.astype(np.float32).astype(BF16),
        b_bcast=np.broadcast_to(b_b.astype(np.float32).astype(BF16), (128, HC)).copy(),
        W2cat=W2cat.astype(np.float32).astype(BF16),
        c2b=np.broadcast_to(c2.astype(np.float32), (128, Fout + 2)).copy(),
        b2c=np.broadcast_to(b2.astype(np.float32), (128, Fout)).copy(),
        identb=np.eye(128, dtype=np.float32).astype(BF16),
    )


# ---------------------------------------------------------------- kernel 1
def build_kernel_1(cfg):
    """Own-node transform: htab[r] = xtp[:,r]^T @ W1cat2."""
    nblk, nrows = cfg["nblk"], cfg["nrows"]
    nc = bacc.Bacc("TRN2", target_bir_lowering=False, debug=False)
    xtp_d = nc.dram_tensor("xTP", [128, nrows], BF, kind="ExternalInput")
    w1_d = nc.dram_tensor("W1cat2", [128, KOUT], BF, kind="ExternalInput")
    htab = nc.dram_tensor("htab", [nrows, KOUT], BF, kind="ExternalOutput")

    with TileContext(nc) as tc:
        with tc.tile_pool(name="consts", bufs=1) as cp:
            xtp = cp.tile([128, nrows], BF)
            nc.sync.dma_start(out=xtp[:], in_=xtp_d[:])
            w1c = cp.tile([128, KOUT], BF)
            nc.sync.dma_start(out=w1c[:], in_=w1_d[:])
            with tc.tile_pool(name="t", bufs=4) as ap, \
                 tc.tile_pool(name="ps", bufs=4, space="PSUM") as aps:
                MB = 8
                for b0 in range(0, nblk, MB):
                    nb = min(MB, nblk - b0)
                    st = ap.tile([128, MB * KOUT], BF, tag="st")
                    for bi in range(nb):
                        b = b0 + bi
                        ps = aps.tile([128, KOUT], F32, tag="ps")
                        nc.tensor.matmul(ps[:], lhsT=xtp[:, b * 128:(b + 1) * 128],
                                         rhs=w1c[:], start=True, stop=True)
                        if bi % 2 == 0:
                            nc.vector.tensor_copy(
                                out=st[:, bi * KOUT:(bi + 1) * KOUT], in_=ps[:])
                        else:
                            nc.scalar.copy(
                                out=st[:, bi * KOUT:(bi + 1) * KOUT], in_=ps[:])
                    dv = htab[b0 * 128:(b0 + nb) * 128, :] \
                        .rearrange("(b p) c -> p b c", p=128)
                    sv = st[:, 0:nb * KOUT].rearrange("p (b c) -> p b c", c=KOUT)
                    nc.sync.dma_start(out=dv, in_=sv)
    nc.finalize()
    return nc


# ---------------------------------------------------------------- kernel 2
def build_kernel_2(cfg, g):
    """Layer-1 edge stage on host-materialized slot payloads (no gathers)."""
    HC, H, Fout = cfg["HC"], cfg["H"], cfg["Fout"]
    nblk, nrows = cfg["nblk"], cfg["nrows"]
    Lu, offs, total_cols = g["Lu"], g["offs"], g["total_cols"]

    nc = bacc.Bacc("TRN2", target_bir_lowering=False, debug=False)
    hg_d = nc.dram_tensor("hg", [128, total_cols * W1CH], BF, kind="ExternalInput")
    ad_d = nc.dram_tensor("adall", [128, nblk * H], BF, kind="ExternalInput")
    bb_d = nc.dram_tensor("b_bcast", [128, HC], BF, kind="ExternalInput")
    w2_d = nc.dram_tensor("W2cat", [128, Fout + 2], BF, kind="ExternalInput")
    c2_d = nc.dram_tensor("c2b", [128, Fout + 2], F32, kind="ExternalInput")
    id_d = nc.dram_tensor("identb", [128, 128], BF, kind="ExternalInput")
    shard = nc.dram_tensor("shard", [nrows, Fout + 2], F32, kind="ExternalOutput")
    Lmax = int(Lu.max())

    with TileContext(nc) as tc:
        with tc.tile_pool(name="consts", bufs=1) as cp:
            adall = cp.tile([128, nblk * H], BF)
            nc.sync.dma_start(out=adall[:], in_=ad_d[:])
            bb = cp.tile([128, HC], BF)
            nc.sync.dma_start(out=bb[:], in_=bb_d[:])
            w2c = cp.tile([128, Fout + 2], BF)
            nc.sync.dma_start(out=w2c[:], in_=w2_d[:])
            c2b = cp.tile([128, Fout + 2], F32)
            nc.sync.dma_start(out=c2b[:], in_=c2_d[:])
            idb = cp.tile([128, 128], BF)
            nc.sync.dma_start(out=idb[:], in_=id_d[:])

            with tc.tile_pool(name="e2", bufs=4) as ep, \
                 tc.tile_pool(name="e2g", bufs=3) as gp, \
                 tc.tile_pool(name="e2m", bufs=3) as mp, \
                 tc.tile_pool(name="e2ps", bufs=3, space="PSUM") as eps, \
                 tc.tile_pool(name="e2ps2", bufs=2, space="PSUM") as eps2:
                # software pipeline: pre+scatter for block i; PSUM-dependent
                # epilogue for block i-1; transpose/W2 for block i-2 — so no
                # in-order engine queue ever waits on an unfinished producer.
                c1 = None
                c2 = None
                for i in range(nblk + 2):
                    if i < nblk:
                        b = i
                        lt = int(Lu[b])
                        off = int(offs[b])
                        gt = gp.tile([128, Lmax * W1CH], BF, tag="g")
                        nc.sync.dma_start(
                            out=gt[:, 0:lt * W1CH],
                            in_=hg_d[:, off * W1CH:(off + lt) * W1CH])
                        gv = gt[:, 0:lt * W1CH].rearrange("p (l w) -> p l w", w=W1CH)
                        adb = adall[:, b * H:(b + 1) * H] \
                            .unsqueeze(1).to_broadcast([128, lt, H])
                        # e = a_s + a_d ; p = exp(leaky_relu(e))
                        e = ep.tile([128, lt * H], BF, tag="e")
                        nc.vector.tensor_tensor(
                            out=e[:].rearrange("p (l h) -> p l h", h=H),
                            in0=gv[:, :, HC:W1CH], in1=adb, op=mybir.AluOpType.add)
                        ab = ep.tile([128, lt * H], BF, tag="ab")
                        nc.scalar.activation(out=ab[:], in_=e[:],
                                             func=mybir.ActivationFunctionType.Abs,
                                             scale=(1.0 - NEG_SLOPE) / (1.0 + NEG_SLOPE))
                        w = ep.tile([128, lt * H], BF, tag="w")
                        nc.gpsimd.tensor_add(out=w[:], in0=e[:], in1=ab[:])
                        p = ep.tile([128, lt * H], BF, tag="p")
                        nc.scalar.activation(out=p[:], in_=w[:],
                                             func=mybir.ActivationFunctionType.Exp,
                                             scale=(1.0 + NEG_SLOPE) / 2.0)
                    else:
                        b = None
                    # ---- epilogue for block i-1 (fills the p-wait gap)
                    if c1 is not None:
                        pso1, rden1, zzt = c1["pso"], c1["rden"], c1["zz"]
                        v0 = ep.tile([128, HC], BF, tag="v0")
                        rexp = rden1[:].unsqueeze(1).to_broadcast([128, HC // H, H])
                        nc.vector.tensor_tensor(
                            out=v0[:].rearrange("p (c h) -> p c h", h=H),
                            in0=pso1[:].rearrange("p (c h) -> p c h", h=H),
                            in1=rexp, op=mybir.AluOpType.mult)
                        v = ep.tile([128, HC], BF, tag="v")
                        nc.gpsimd.tensor_add(out=v[:], in0=v0[:], in1=bb[:])
                        rr = ep.tile([128, HC], BF, tag="rr")
                        nc.scalar.activation(out=rr[:], in_=v[:],
                                             func=mybir.ActivationFunctionType.Relu)
                        mn = ep.tile([128, HC], BF, tag="mn")
                        nc.gpsimd.tensor_tensor(out=mn[:], in0=v[:], in1=rr[:],
                                                op=mybir.AluOpType.subtract)
                        u = ep.tile([128, HC], BF, tag="u")
                        nc.scalar.activation(out=u[:], in_=mn[:],
                                             func=mybir.ActivationFunctionType.Exp)
                        nc.gpsimd.tensor_add(out=zzt[:], in0=rr[:], in1=u[:])
                    # ---- layer-2 transform for block i-2
                    if c2 is not None:
                        pso2, zz2, b2i = c2["pso"], c2["zz"], c2["b"]
                        pst = eps2.tile([128, 128], BF, tag="pst")
                        nc.tensor.transpose(out=pst[:], in_=zz2[:], identity=idb[:])
                        zt = ep.tile([128, 128], BF, tag="zt")
                        nc.scalar.copy(out=zt[:], in_=pst[:])
                        ph = eps2.tile([128, Fout + 2], F32, tag="ph")
                        nc.tensor.matmul(ph[:], lhsT=zt[:], rhs=w2c[:],
                                         start=True, stop=True)
                        h2a = ep.tile([128, Fout + 2], F32, tag="h2a")
                        nc.vector.tensor_tensor(out=h2a[:], in0=ph[:], in1=c2b[:],
                                                op=mybir.AluOpType.subtract)
                        nc.sync.dma_start(out=shard[b2i * 128:(b2i + 1) * 128, :],
                                          in_=h2a[:])
                    # ---- rest of pre + scatter for block i
                    if i < nblk:
                        den = ep.tile([128, H], F32, tag="den")
                        nc.vector.tensor_reduce(
                            out=den[:], in_=p[:].rearrange("p (l h) -> p h l", h=H),
                            axis=mybir.AxisListType.X, op=mybir.AluOpType.add)
                        rden = ep.tile([128, H], F32, tag="rden")
                        nc.vector.reciprocal(out=rden[:], in_=den[:])
                        m = mp.tile([128, Lmax * HC], BF, tag="m")
                        hview = gv[:, :, 0:HC].rearrange("p l (c h) -> p l c h", h=H)
                        pexp = p[:].rearrange("p (l h) -> p l h", h=H) \
                            .unsqueeze(2).to_broadcast([128, lt, HC // H, H])
                        nc.vector.tensor_tensor(
                            out=m[:, 0:lt * HC].rearrange(
                                "p (l c h) -> p l c h", c=HC // H, h=H),
                            in0=hview, in1=pexp, op=mybir.AluOpType.mult)
                        pso = eps.tile([128, HC], F32, tag="pso")
                        for j in range(lt):
                            nc.tensor.matmul(pso[:], lhsT=idb[:],
                                             rhs=m[:, j * HC:(j + 1) * HC],
                                             start=(j == 0), stop=(j == lt - 1))
                        zzn = ep.tile([128, HC], BF, tag="zz")
                        nxt = {"pso": pso, "rden": rden, "zz": zzn, "b": b}
                    else:
                        nxt = None
                    c2 = c1
                    c1 = nxt
    nc.finalize()
    return nc


# ---------------------------------------------------------------- kernel 3
def build_kernel_3(cfg, g):
    """Layer-2 edge stage (H=1) + log_softmax with batched ln()."""
    Fout = cfg["Fout"]
    nblk = cfg["nblk"]
    Lu, offs, total_cols = g["Lu"], g["offs"], g["total_cols"]

    nc = bacc.Bacc("TRN2", target_bir_lowering=False, debug=False)
    hg_d = nc.dram_tensor("hg2", [128, total_cols * W2CH], F32, kind="ExternalInput")
    ad_d = nc.dram_tensor("ad2all", [128, nblk], F32, kind="ExternalInput")
    b2_d = nc.dram_tensor("b2c", [128, Fout], F32, kind="ExternalInput")
    id_d = nc.dram_tensor("identb", [128, 128], BF, kind="ExternalInput")
    outsh = nc.dram_tensor("outsh", [128, nblk * Fout], F32, kind="ExternalOutput")
    Lmax = int(Lu.max())

    with TileContext(nc) as tc:
        with tc.tile_pool(name="consts", bufs=1) as cp:
            ad2 = cp.tile([128, nblk], F32)
            nc.sync.dma_start(out=ad2[:], in_=ad_d[:])
            b2c = cp.tile([128, Fout], F32)
            nc.sync.dma_start(out=b2c[:], in_=b2_d[:])
            idb = cp.tile([128, 128], BF)
            nc.sync.dma_start(out=idb[:], in_=id_d[:])
            obuf = cp.tile([128, nblk * Fout], F32)
            sebuf = cp.tile([128, nblk], F32)

            with tc.tile_pool(name="e3", bufs=6) as ep, \
                 tc.tile_pool(name="e3g", bufs=3) as gp, \
                 tc.tile_pool(name="e3m", bufs=3) as mp, \
                 tc.tile_pool(name="e3ps", bufs=3, space="PSUM") as eps:
                c1 = None
                for i in range(nblk + 1):
                    if i < nblk:
                        b = i
                        lt = int(Lu[b])
                        off = int(offs[b])
                        gt = gp.tile([128, Lmax * W2CH], F32, tag="g")
                        nc.sync.dma_start(
                            out=gt[:, 0:lt * W2CH],
                            in_=hg_d[:, off * W2CH:(off + lt) * W2CH])
                        gv = gt[:, 0:lt * W2CH].rearrange("p (l w) -> p l w", w=W2CH)
                        adb = ad2[:, b:b + 1].to_broadcast([128, lt])
                        e2 = ep.tile([128, lt], F32, tag="e2")
                        nc.vector.tensor_tensor(out=e2[:],
                                                in0=gv[:, :, Fout:Fout + 1].squeeze(),
                                                in1=adb, op=mybir.AluOpType.add)
                        ab2 = ep.tile([128, lt], F32, tag="ab2")
                        nc.scalar.activation(out=ab2[:], in_=e2[:],
                                             func=mybir.ActivationFunctionType.Abs,
                                             scale=(1.0 - NEG_SLOPE) / (1.0 + NEG_SLOPE))
                        w2t = ep.tile([128, lt], F32, tag="w2t")
                        nc.gpsimd.tensor_add(out=w2t[:], in0=e2[:], in1=ab2[:])
                        # p2 = exp(leaky(e2)); den2 comes free via accum (H=1)
                        p2 = ep.tile([128, lt], F32, tag="p2")
                        den2 = ep.tile([128, 1], F32, tag="den2")
                        nc.scalar.activation(out=p2[:], in_=w2t[:],
                                             func=mybir.ActivationFunctionType.Exp,
                                             scale=(1.0 + NEG_SLOPE) / 2.0,
                                             accum_out=den2[:])
                    # ---- epilogue for block i-1 (fills the p2-wait gap)
                    if c1 is not None:
                        ps21, rden21, b1i = c1["ps2"], c1["rden2"], c1["b"]
                        o2 = ep.tile([128, Fout], F32, tag="o2")
                        r2e = rden21[:].to_broadcast([128, Fout])
                        nc.vector.tensor_tensor(out=o2[:], in0=ps21[:], in1=r2e,
                                                op=mybir.AluOpType.mult)
                        # o3 written straight into the persistent output buffer;
                        # log-softmax runs without a max-shift (f32-safe range)
                        ob = obuf[:, b1i * Fout:(b1i + 1) * Fout]
                        nc.vector.tensor_tensor(out=ob, in0=o2[:], in1=b2c[:],
                                                op=mybir.AluOpType.add)
                        ex = ep.tile([128, Fout], F32, tag="ex")
                        nc.scalar.activation(out=ex[:], in_=ob,
                                             func=mybir.ActivationFunctionType.Exp,
                                             accum_out=sebuf[:, b1i:b1i + 1])
                    if i < nblk:
                        rden2 = ep.tile([128, 1], F32, tag="rden2")
                        nc.vector.reciprocal(out=rden2[:], in_=den2[:])
                        m2 = mp.tile([128, Lmax * Fout], BF, tag="m2")
                        p2e = p2[:].unsqueeze(2).to_broadcast([128, lt, Fout])
                        nc.vector.tensor_tensor(
                            out=m2[:, 0:lt * Fout].rearrange("p (l f) -> p l f", f=Fout),
                            in0=gv[:, :, 0:Fout], in1=p2e, op=mybir.AluOpType.mult)
                        ps2 = eps.tile([128, Fout], F32, tag="ps2")
                        for j in range(lt):
                            nc.tensor.matmul(ps2[:], lhsT=idb[:],
                                             rhs=m2[:, j * Fout:(j + 1) * Fout],
                                             start=(j == 0), stop=(j == lt - 1))
                        c1 = {"ps2": ps2, "rden2": rden2, "b": b}
                    else:
                        c1 = None
                # batched ln over all blocks, then one fused subtract + DMA out
                ls = cp.tile([128, nblk], F32)
                nc.scalar.activation(out=ls[:], in_=sebuf[:],
                                     func=mybir.ActivationFunctionType.Ln)
                ov = cp.tile([128, nblk * Fout], F32)
                lsv = ls[:].unsqueeze(2).to_broadcast([128, nblk, Fout])
                nc.vector.tensor_tensor(
                    out=ov[:].rearrange("p (b f) -> p b f", f=Fout),
                    in0=obuf[:].rearrange("p (b f) -> p b f", f=Fout),
                    in1=lsv, op=mybir.AluOpType.subtract)
                nc.sync.dma_start(out=outsh[:], in_=ov[:])
    nc.finalize()
    return nc


# ---------------------------------------------------------------- runner
_TRACE = False
last_times = {}


def _run_spmd(nc, in_maps, ncores):
    kw = {}
    if _TRACE:
        _install_hook()
        kw["trace"] = True
    return bass_utils.run_bass_kernel_spmd(nc, in_maps, core_ids=list(range(ncores)), **kw)


def _install_hook():
    try:
        import antenv
        if "antenv.axon_hooks" not in sys.modules:
            hooks_mod = types.ModuleType("antenv.axon_hooks")
            _h = [None]
            hooks_mod.set_axon_ntff_profile_hook = lambda h: _h.__setitem__(0, h)
            hooks_mod.get_axon_ntff_profile_hook = lambda: _h[0]
            sys.modules["antenv.axon_hooks"] = hooks_mod
            antenv.axon_hooks = hooks_mod
            from trn_agent_boot.trn_boot import _ntff_profile_via_ctypes
            hooks_mod.set_axon_ntff_profile_hook(
                _ntff_profile_via_ctypes('/opt/axon/libaxon_pjrt.so'))
    except Exception as e:  # pragma: no cover
        print("hook install failed:", e, file=sys.stderr)


def gat_forward(cfg, inputs):
    N, Fout, H, HC = cfg["N"], cfg["Fout"], cfg["H"], cfg["HC"]
    ncores, npc, nrows, nblk = cfg["ncores"], cfg["npc"], cfg["nrows"], cfg["nblk"]
    x = np.asarray(inputs["x"], np.float32)
    edge_index = np.asarray(inputs["edge_index"])

    g = preprocess_graph(cfg, edge_index)
    pp = preprocess_params(cfg, *[np.asarray(inputs[k]) for k in
                                  ("W1", "att_src1", "att_dst1", "b1", "bn_gamma",
                                   "bn_beta", "bn_mean", "bn_var", "W2",
                                   "att_src2", "att_dst2", "b2")])

    # ---- K1: per-core own-node transform
    nc1 = build_kernel_1(cfg)
    in1 = []
    for k in range(ncores):
        c = g["cores"][k]
        xtp = np.zeros((128, nrows), np.float32)
        valid = c["row2node"] >= 0
        xtp[:, valid] = x[c["row2node"][valid]].T
        in1.append({"xTP": xtp.astype(BF16), "W1cat2": pp["W1cat2"]})
    res1 = _run_spmd(nc1, in1, ncores)
    last_times["K1"] = res1.exec_time_ns

    # ---- host: assemble h|a_s table + per-core a_d, materialize slots
    htab_all = np.zeros((N + 1, KOUT), BF16)
    htab_all[N, HC:HC + H] = BF16(-1e30)           # sentinel: a_s = -inf
    for k in range(ncores):
        sh = res1.results[k]["htab"]
        c = g["cores"][k]
        valid = c["row2node"] >= 0
        htab_all[c["row2node"][valid]] = sh[valid]
    hg_cores = materialize_slots(cfg, g, htab_all[:, :W1CH], W1CH)

    nc2 = build_kernel_2(cfg, g)
    in2 = []
    for k in range(ncores):
        c = g["cores"][k]
        adall = np.zeros((128, nblk * H), BF16)
        r2n = c["row2node"].reshape(nblk, 128)
        for b in range(nblk):
            vb = r2n[b] >= 0
            adall[vb, b * H:(b + 1) * H] = htab_all[r2n[b][vb], HC + H:KOUT]
        in2.append({"hg": hg_cores[k], "adall": adall,
                    "b_bcast": pp["b_bcast"], "W2cat": pp["W2cat"],
                    "c2b": pp["c2b"], "identb": pp["identb"]})
    res2 = _run_spmd(nc2, in2, ncores)
    last_times["K2"] = res2.exec_time_ns

    # ---- host: assemble layer-2 table, materialize slots (f32)
    h2a_all = np.zeros((N + 1, W2CH), np.float32)
    h2a_all[N, Fout] = -1e30                       # sentinel: a_s2 = -inf
    for k in range(ncores):
        sh = res2.results[k]["shard"]
        c = g["cores"][k]
        valid = c["row2node"] >= 0
        h2a_all[c["row2node"][valid]] = sh[valid]
    hg2_cores = materialize_slots(cfg, g, h2a_all, W2CH)

    nc3 = build_kernel_3(cfg, g)
    in3 = []
    for k in range(ncores):
        c = g["cores"][k]
        ad2all = np.zeros((128, nblk), np.float32)
        r2n = c["row2node"].reshape(nblk, 128)
        for b in range(nblk):
            vb = r2n[b] >= 0
            ad2all[vb, b] = h2a_all[r2n[b][vb], Fout + 1]
        in3.append({"hg2": hg2_cores[k], "ad2all": ad2all,
                    "b2c": pp["b2c"], "identb": pp["identb"]})
    res3 = _run_spmd(nc3, in3, ncores)
    last_times["K3"] = res3.exec_time_ns

    out = np.zeros((N, Fout), np.float32)
    for k in range(ncores):
        sh = res3.results[k]["outsh"]              # [128, nblk*Fout]
        c = g["cores"][k]
        vals = sh.reshape(128, nblk, Fout).transpose(1, 0, 2).reshape(nrows, Fout)
        valid = c["row2node"] >= 0
        out[c["row2node"][valid]] = vals[valid]
    return out


def kernel(**inputs):
    cfg = make_cfg()
    return gat_forward(cfg, inputs)


# revision 16
# speedup vs baseline: 1.4160x; 1.1801x over previous
"""GAT (2-layer, PyG-style) on 8 Trainium2 NeuronCores.

Strategy (dst-owner sharding, gather-free):
  - Nodes partitioned across 8 cores by dst id; every explicit edge plus one
    self-loop per node becomes a slot in a padded-CSR layout (128 dst rows
    per block, block slot-count L_b unified across cores for SPMD).
  - K1 (per core): transform own nodes h|a_s|a_d = x @ [W1*bn | As | Ad]
    -> htab shard (bf16, block-permuted order).
  - Host: concat shards, materialize the per-slot edge payload (h|a_s of the
    src node of every slot) in a partition-blocked sequential layout, so the
    edge kernels need no dma_gather (the Q7 descriptor-generation bottleneck
    of gather-based variants) — every DMA is a plain contiguous HWDGE read.
  - K2 (per core): per dst-block: sequential DMA of slot payloads,
    segment-softmax attention (denominator folded at the end), PSUM identity-
    matmul scatter, fused BN+ELU, layer-2 input transform -> h2|a_s2|a_d2.
  - Host: assemble + materialize layer-2 per-slot payload (f32).
  - K3 (per core): same edge stage with H=1, log_softmax with the ln() batched
    over all blocks at the end (avoids per-block activation-table reloads).
  - Host: un-permute rows, concat cores.
"""
import sys
import types

sys.path.insert(0, "/opt/trn_rl_repo")

import numpy as np
import ml_dtypes

BF16 = ml_dtypes.bfloat16

import concourse.bacc as bacc
import concourse.bass as bass
import concourse.mybir as mybir
from concourse.tile import TileContext
from concourse import bass_utils

F32 = mybir.dt.float32
BF = mybir.dt.bfloat16

NEG_SLOPE = 0.2
BN_EPS = 1e-5

W1CH = 136          # per-slot layer-1 payload elems (h 128 | a_s 8), bf16
W2CH = 42           # per-slot layer-2 payload elems (h2 40 | a_s2 | a_d2), f32
KOUT = 144          # K1 output row (h 128 | a_s 8 | a_d 8)


# ---------------------------------------------------------------- config
def make_cfg(N=50000, E=800000, Fin=128, H=8, C1=16, Fout=40, ncores=8):
    cfg = {}
    cfg["N"], cfg["E"] = N, E
    cfg["Fin"], cfg["H"], cfg["C1"], cfg["Fout"] = Fin, H, C1, Fout
    cfg["HC"] = H * C1
    cfg["ncores"] = ncores
    assert N % ncores == 0
    cfg["npc"] = N // ncores                       # nodes per core
    cfg["nblk"] = (cfg["npc"] + 127) // 128        # dst blocks per core
    cfg["nrows"] = cfg["nblk"] * 128               # shard rows (padded)
    assert Fin == 128 and cfg["HC"] == 128
    return cfg


# ------------------------------------------------------------ host graph prep
def preprocess_graph(cfg, edge_index):
    """Per-core padded-CSR slot structure (self-loops included as slots)."""
    N, ncores, npc = cfg["N"], cfg["ncores"], cfg["npc"]
    nblk, nrows = cfg["nblk"], cfg["nrows"]
    src = np.asarray(edge_index[0], np.int64)
    dst = np.asarray(edge_index[1], np.int64)

    cores = []
    for k in range(ncores):
        m = (dst // npc) == k
        own = np.arange(npc, dtype=np.int64)
        s_k = np.concatenate([src[m], own + k * npc])     # + self-loops
        d_loc = np.concatenate([dst[m] - k * npc, own])
        deg = np.bincount(d_loc, minlength=npc)
        order = np.argsort(-deg, kind="stable")
        row2node = np.full(nrows, -1, np.int64)
        row2node[:npc] = order + k * npc
        fin_rank = np.full(npc, -1, np.int64)
        fin_rank[order] = np.arange(npc)
        degs = deg[order]
        L = np.zeros(nblk, np.int64)
        for b in range(nblk):
            sl = slice(b * 128, min((b + 1) * 128, npc))
            L[b] = max(1, int(degs[sl].max())) if sl.start < npc else 1
        cores.append(dict(s_k=s_k, d_loc=d_loc, row2node=row2node,
                          fin_rank=fin_rank, L=L))

    # unify per-block slot counts across cores (blocks already deg-sorted)
    Lu = np.zeros(nblk, np.int64)
    for c in cores:
        Lu = np.maximum(Lu, c["L"])
    offs = np.zeros(nblk + 1, np.int64)
    offs[1:] = np.cumsum(Lu)
    total_cols = int(offs[-1])

    # slot_src[b]: [Lu[b], 128] global src node id, -1 = pad
    for c in cores:
        re = c["fin_rank"][c["d_loc"]]
        okey = np.argsort(re, kind="stable")
        rr = re[okey]
        ss = c["s_k"][okey]
        jj = np.arange(len(rr)) - np.searchsorted(rr, rr, side="left")
        slot_src = [np.full((int(Lu[b]), 128), -1, np.int64) for b in range(nblk)]
        b_e = rr // 128
        p_e = rr % 128
        for b in range(nblk):
            sel = b_e == b
            slot_src[b][jj[sel], p_e[sel]] = ss[sel]
        c["slot_src"] = slot_src

    return dict(cores=cores, Lu=Lu, offs=offs, total_cols=total_cols)


def materialize_slots(cfg, g, tab_ext, W):
    """tab_ext: [N+1, W] payload per node (+ sentinel row N).
    Returns per-core [128, total_cols*W] partition-blocked slot payload."""
    nblk = cfg["nblk"]
    N = cfg["N"]
    out = []
    for c in g["cores"]:
        parts = []
        for b in range(nblk):
            sl = c["slot_src"][b]                      # [L, 128]
            idx = np.where(sl >= 0, sl, N)
            pay = tab_ext[idx]                         # [L, 128, W]
            parts.append(np.ascontiguousarray(pay.transpose(1, 0, 2))
                         .reshape(128, -1))
        out.append(np.concatenate(parts, axis=1))
    return out


# ------------------------------------------------------------ host param prep
def preprocess_params(cfg, W1, att_src1, att_dst1, b1, bn_gamma, bn_beta,
                      bn_mean, bn_var, W2, att_src2, att_dst2, b2):
    H, C1v, HC, Fout = cfg["H"], cfg["C1"], cfg["HC"], cfg["Fout"]
    W1 = W1.astype(np.float64)
    W2 = W2.astype(np.float64)
    a_feat = bn_gamma.astype(np.float64) / np.sqrt(bn_var.astype(np.float64) + BN_EPS)
    b_feat = (b1.astype(np.float64) - bn_mean.astype(np.float64)) * a_feat \
        + bn_beta.astype(np.float64)
    As = np.zeros((HC, H))
    Ad = np.zeros((HC, H))
    for h in range(H):
        As[h * C1v:(h + 1) * C1v, h] = att_src1[h].astype(np.float64)
        Ad[h * C1v:(h + 1) * C1v, h] = att_dst1[h].astype(np.float64)
    As_eff = W1 @ As
    Ad_eff = W1 @ Ad
    colmap = np.array([h * C1v + c for c in range(C1v) for h in range(H)])
    W1a_r = (W1 * a_feat[None, :])[:, colmap]
    W1cat2 = np.concatenate([W1a_r, As_eff, Ad_eff], axis=1)  # [Fin, 152]
    b_b = b_feat[colmap]
    w_s2 = W2 @ att_src2[0].astype(np.float64)
    w_d2 = W2 @ att_dst2[0].astype(np.float64)
    W2cat = np.concatenate([W2, w_s2[:, None], w_d2[:, None]], axis=1)[colmap, :]
    c2 = W2cat.sum(axis=0)                                    # [Fout+2]
    return dict(
        W1cat2=W1cat2.astype(np.float32).astype(BF16),
        b_bcast=np.broadcast_to(b_b.astype(np.float32).astype(BF16), (128, HC)).copy(),
        W2cat=W2cat.astype(np.float32).astype(BF16),
        c2b=np.broadcast_to(c2.astype(np.float32), (128, Fout + 2)).copy(),
        b2c=np.broadcast_to(b2.astype(np.float32), (128, Fout)).copy(),
        identb=np.eye(128, dtype=np.float32).astype(BF16),
    )


# ---------------------------------------------------------------- kernel 1
def build_kernel_1(cfg):
    """Own-node transform: htab[r] = xtp[:,r]^T @ W1cat2."""
    nblk, nrows = cfg["nblk"], cfg["nrows"]
    nc = bacc.Bacc("TRN2", target_bir_lowering=False, debug=False)
    xtp_d = nc.dram_tensor("xTP", [128, nrows], BF, kind="ExternalInput")
    w1_d = nc.dram_tensor("W1cat2", [128, KOUT], BF, kind="ExternalInput")
    htab = nc.dram_tensor("htab", [nrows, KOUT], BF, kind="ExternalOutput")

    with TileContext(nc) as tc:
        with tc.tile_pool(name="consts", bufs=1) as cp:
            xtp = cp.tile([128, nrows], BF)
            nc.sync.dma_start(out=xtp[:], in_=xtp_d[:])
            w1c = cp.tile([128, KOUT], BF)
            nc.sync.dma_start(out=w1c[:], in_=w1_d[:])
            with tc.tile_pool(name="t", bufs=4) as ap, \
                 tc.tile_pool(name="ps", bufs=4, space="PSUM") as aps:
                MB = 8
                for b0 in range(0, nblk, MB):
                    nb = min(MB, nblk - b0)
                    st = ap.tile([128, MB * KOUT], BF, tag="st")
                    for bi in range(nb):
                        b = b0 + bi
                        ps = aps.tile([128, KOUT], F32, tag="ps")
                        nc.tensor.matmul(ps[:], lhsT=xtp[:, b * 128:(b + 1) * 128],
                                         rhs=w1c[:], start=True, stop=True)
                        if bi % 2 == 0:
                            nc.vector.tensor_copy(
                                out=st[:, bi * KOUT:(bi + 1) * KOUT], in_=ps[:])
                        else:
                            nc.scalar.copy(
                                out=st[:, bi * KOUT:(bi + 1) * KOUT], in_=ps[:])
                    dv = htab[b0 * 128:(b0 + nb) * 128, :] \
                        .rearrange("(b p) c -> p b c", p=128)
                    sv = st[:, 0:nb * KOUT].rearrange("p (b c) -> p b c", c=KOUT)
                    nc.sync.dma_start(out=dv, in_=sv)
    nc.finalize()
    return nc


# ---------------------------------------------------------------- kernel 2
def build_kernel_2(cfg, g):
    """Layer-1 edge stage on host-materialized slot payloads (no gathers)."""
    HC, H, Fout = cfg["HC"], cfg["H"], cfg["Fout"]
    nblk, nrows = cfg["nblk"], cfg["nrows"]
    Lu, offs, total_cols = g["Lu"], g["offs"], g["total_cols"]

    nc = bacc.Bacc("TRN2", target_bir_lowering=False, debug=False)
    hg_d = nc.dram_tensor("hg", [128, total_cols * W1CH], BF, kind="ExternalInput")
    ad_d = nc.dram_tensor("adall", [128, nblk * H], BF, kind="ExternalInput")
    bb_d = nc.dram_tensor("b_bcast", [128, HC], BF, kind="ExternalInput")
    w2_d = nc.dram_tensor("W2cat", [128, Fout + 2], BF, kind="ExternalInput")
    c2_d = nc.dram_tensor("c2b", [128, Fout + 2], F32, kind="ExternalInput")
    id_d = nc.dram_tensor("identb", [128, 128], BF, kind="ExternalInput")
    shard = nc.dram_tensor("shard", [nrows, Fout + 2], F32, kind="ExternalOutput")
    Lmax = int(Lu.max())

    with TileContext(nc) as tc:
        with tc.tile_pool(name="consts", bufs=1) as cp:
            adall = cp.tile([128, nblk * H], BF)
            nc.sync.dma_start(out=adall[:], in_=ad_d[:])
            bb = cp.tile([128, HC], BF)
            nc.sync.dma_start(out=bb[:], in_=bb_d[:])
            w2c = cp.tile([128, Fout + 2], BF)
            nc.sync.dma_start(out=w2c[:], in_=w2_d[:])
            c2b = cp.tile([128, Fout + 2], F32)
            nc.sync.dma_start(out=c2b[:], in_=c2_d[:])
            idb = cp.tile([128, 128], BF)
            nc.sync.dma_start(out=idb[:], in_=id_d[:])

            with tc.tile_pool(name="e2", bufs=4) as ep, \
                 tc.tile_pool(name="e2g", bufs=4) as gp, \
                 tc.tile_pool(name="e2m", bufs=3) as mp, \
                 tc.tile_pool(name="e2ps", bufs=3, space="PSUM") as eps, \
                 tc.tile_pool(name="e2ps2", bufs=2, space="PSUM") as eps2:
                # 5-deep software pipeline: DMA block t | attention front
                # t-1 | messages+scatter t-2 | BN/ELU epilogue t-3 | layer-2
                # transform t-4.  Every op is ready when its in-order engine
                # queue reaches it, so no engine ever stalls on a producer.
                S = [dict() for _ in range(nblk)]
                for t in range(nblk + 6):
                    if t < nblk:
                        b = t
                        st = S[b]
                        lt = int(Lu[b])
                        off = int(offs[b])
                        gt = gp.tile([128, Lmax * W1CH], BF, tag="g")
                        nc.sync.dma_start(
                            out=gt[:, 0:lt * W1CH],
                            in_=hg_d[:, off * W1CH:(off + lt) * W1CH])
                        st["gt"] = gt
                        st["lt"] = lt
                    # ---- attention front for block t-1
                    if 0 <= t - 1 < nblk:
                        b = t - 1
                        st = S[b]
                        lt = st["lt"]
                        gv = st["gt"][:, 0:lt * W1CH].rearrange(
                            "p (l w) -> p l w", w=W1CH)
                        adb = adall[:, b * H:(b + 1) * H] \
                            .unsqueeze(1).to_broadcast([128, lt, H])
                        e = ep.tile([128, lt * H], BF, tag="e")
                        nc.vector.tensor_tensor(
                            out=e[:].rearrange("p (l h) -> p l h", h=H),
                            in0=gv[:, :, HC:W1CH], in1=adb, op=mybir.AluOpType.add)
                        ab = ep.tile([128, lt * H], BF, tag="ab")
                        nc.scalar.activation(out=ab[:], in_=e[:],
                                             func=mybir.ActivationFunctionType.Abs,
                                             scale=(1.0 - NEG_SLOPE) / (1.0 + NEG_SLOPE))
                        w = ep.tile([128, lt * H], BF, tag="w")
                        nc.gpsimd.tensor_add(out=w[:], in0=e[:], in1=ab[:])
                        p = ep.tile([128, lt * H], BF, tag="p")
                        nc.scalar.activation(out=p[:], in_=w[:],
                                             func=mybir.ActivationFunctionType.Exp,
                                             scale=(1.0 + NEG_SLOPE) / 2.0)
                        st["p"] = p
                    # ---- messages + scatter for block t-2
                    if 0 <= t - 2 < nblk:
                        b = t - 2
                        st = S[b]
                        lt = st["lt"]
                        p = st["p"]
                        gv = st["gt"][:, 0:lt * W1CH].rearrange(
                            "p (l w) -> p l w", w=W1CH)
                        den = ep.tile([128, H], F32, tag="den")
                        nc.vector.tensor_reduce(
                            out=den[:], in_=p[:].rearrange("p (l h) -> p h l", h=H),
                            axis=mybir.AxisListType.X, op=mybir.AluOpType.add)
                        rden = ep.tile([128, H], F32, tag="rden")
                        nc.vector.reciprocal(out=rden[:], in_=den[:])
                        m = mp.tile([128, Lmax * HC], BF, tag="m")
                        hview = gv[:, :, 0:HC].rearrange("p l (c h) -> p l c h", h=H)
                        pexp = p[:].rearrange("p (l h) -> p l h", h=H) \
                            .unsqueeze(2).to_broadcast([128, lt, HC // H, H])
                        nc.vector.tensor_tensor(
                            out=m[:, 0:lt * HC].rearrange(
                                "p (l c h) -> p l c h", c=HC // H, h=H),
                            in0=hview, in1=pexp, op=mybir.AluOpType.mult)
                        pso = eps.tile([128, HC], F32, tag="pso")
                        for j in range(lt):
                            nc.tensor.matmul(pso[:], lhsT=idb[:],
                                             rhs=m[:, j * HC:(j + 1) * HC],
                                             start=(j == 0), stop=(j == lt - 1))
                        st["pso"] = pso
                        st["rden"] = rden
                    # ---- BN/ELU epilogue for block t-3
                    if 0 <= t - 3 < nblk:
                        b = t - 3
                        st = S[b]
                        pso, rden = st["pso"], st["rden"]
                        v0 = ep.tile([128, HC], BF, tag="v0")
                        rexp = rden[:].unsqueeze(1).to_broadcast([128, HC // H, H])
                        nc.vector.tensor_tensor(
                            out=v0[:].rearrange("p (c h) -> p c h", h=H),
                            in0=pso[:].rearrange("p (c h) -> p c h", h=H),
                            in1=rexp, op=mybir.AluOpType.mult)
                        v = ep.tile([128, HC], BF, tag="v")
                        nc.gpsimd.tensor_add(out=v[:], in0=v0[:], in1=bb[:])
                        rr = ep.tile([128, HC], BF, tag="rr")
                        nc.scalar.activation(out=rr[:], in_=v[:],
                                             func=mybir.ActivationFunctionType.Relu)
                        mn = ep.tile([128, HC], BF, tag="mn")
                        nc.gpsimd.tensor_tensor(out=mn[:], in0=v[:], in1=rr[:],
                                                op=mybir.AluOpType.subtract)
                        u = ep.tile([128, HC], BF, tag="u")
                        nc.scalar.activation(out=u[:], in_=mn[:],
                                             func=mybir.ActivationFunctionType.Exp)
                        zz = ep.tile([128, HC], BF, tag="zz")
                        nc.gpsimd.tensor_add(out=zz[:], in0=rr[:], in1=u[:])
                        st["zz"] = zz
                    # ---- layer-2 transform for block t-5
                    if 0 <= t - 5 < nblk:
                        b = t - 5
                        st = S[b]
                        zz = st["zz"]
                        pst = eps2.tile([128, 128], BF, tag="pst")
                        nc.tensor.transpose(out=pst[:], in_=zz[:], identity=idb[:])
                        zt = ep.tile([128, 128], BF, tag="zt")
                        nc.scalar.copy(out=zt[:], in_=pst[:])
                        ph = eps2.tile([128, Fout + 2], F32, tag="ph")
                        nc.tensor.matmul(ph[:], lhsT=zt[:], rhs=w2c[:],
                                         start=True, stop=True)
                        h2a = ep.tile([128, Fout + 2], F32, tag="h2a")
                        nc.vector.tensor_tensor(out=h2a[:], in0=ph[:], in1=c2b[:],
                                                op=mybir.AluOpType.subtract)
                        nc.sync.dma_start(out=shard[b * 128:(b + 1) * 128, :],
                                          in_=h2a[:])
                        S[b] = {}
    nc.finalize()
    return nc


# ---------------------------------------------------------------- kernel 3
def build_kernel_3(cfg, g):
    """Layer-2 edge stage (H=1) + log_softmax with batched ln()."""
    Fout = cfg["Fout"]
    nblk = cfg["nblk"]
    Lu, offs, total_cols = g["Lu"], g["offs"], g["total_cols"]

    nc = bacc.Bacc("TRN2", target_bir_lowering=False, debug=False)
    hg_d = nc.dram_tensor("hg2", [128, total_cols * W2CH], F32, kind="ExternalInput")
    ad_d = nc.dram_tensor("ad2all", [128, nblk], F32, kind="ExternalInput")
    b2_d = nc.dram_tensor("b2c", [128, Fout], F32, kind="ExternalInput")
    id_d = nc.dram_tensor("identb", [128, 128], BF, kind="ExternalInput")
    outsh = nc.dram_tensor("outsh", [128, nblk * Fout], F32, kind="ExternalOutput")
    Lmax = int(Lu.max())

    with TileContext(nc) as tc:
        with tc.tile_pool(name="consts", bufs=1) as cp:
            ad2 = cp.tile([128, nblk], F32)
            nc.sync.dma_start(out=ad2[:], in_=ad_d[:])
            b2c = cp.tile([128, Fout], F32)
            nc.sync.dma_start(out=b2c[:], in_=b2_d[:])
            idb = cp.tile([128, 128], BF)
            nc.sync.dma_start(out=idb[:], in_=id_d[:])
            obuf = cp.tile([128, nblk * Fout], F32)
            sebuf = cp.tile([128, nblk], F32)

            with tc.tile_pool(name="e3", bufs=6) as ep, \
                 tc.tile_pool(name="e3g", bufs=4) as gp, \
                 tc.tile_pool(name="e3m", bufs=3) as mp, \
                 tc.tile_pool(name="e3ps", bufs=3, space="PSUM") as eps:
                # 4-deep pipeline: DMA t | attention front t-1 |
                # messages+scatter t-2 | epilogue t-3 (see kernel 2).
                S = [dict() for _ in range(nblk)]
                for t in range(nblk + 4):
                    if t < nblk:
                        b = t
                        st = S[b]
                        lt = int(Lu[b])
                        off = int(offs[b])
                        gt = gp.tile([128, Lmax * W2CH], F32, tag="g")
                        nc.sync.dma_start(
                            out=gt[:, 0:lt * W2CH],
                            in_=hg_d[:, off * W2CH:(off + lt) * W2CH])
                        st["gt"] = gt
                        st["lt"] = lt
                    if 0 <= t - 1 < nblk:
                        b = t - 1
                        st = S[b]
                        lt = st["lt"]
                        gv = st["gt"][:, 0:lt * W2CH].rearrange(
                            "p (l w) -> p l w", w=W2CH)
                        adb = ad2[:, b:b + 1].to_broadcast([128, lt])
                        e2 = ep.tile([128, lt], F32, tag="e2")
                        nc.vector.tensor_tensor(out=e2[:],
                                                in0=gv[:, :, Fout:Fout + 1].squeeze(),
                                                in1=adb, op=mybir.AluOpType.add)
                        ab2 = ep.tile([128, lt], F32, tag="ab2")
                        nc.scalar.activation(out=ab2[:], in_=e2[:],
                                             func=mybir.ActivationFunctionType.Abs,
                                             scale=(1.0 - NEG_SLOPE) / (1.0 + NEG_SLOPE))
                        w2t = ep.tile([128, lt], F32, tag="w2t")
                        nc.gpsimd.tensor_add(out=w2t[:], in0=e2[:], in1=ab2[:])
                        # p2 = exp(leaky(e2)); den2 comes free via accum (H=1)
                        p2 = ep.tile([128, lt], F32, tag="p2")
                        den2 = ep.tile([128, 1], F32, tag="den2")
                        nc.scalar.activation(out=p2[:], in_=w2t[:],
                                             func=mybir.ActivationFunctionType.Exp,
                                             scale=(1.0 + NEG_SLOPE) / 2.0,
                                             accum_out=den2[:])
                        st["p2"] = p2
                        st["den2"] = den2
                    if 0 <= t - 2 < nblk:
                        b = t - 2
                        st = S[b]
                        lt = st["lt"]
                        p2, den2 = st["p2"], st["den2"]
                        gv = st["gt"][:, 0:lt * W2CH].rearrange(
                            "p (l w) -> p l w", w=W2CH)
                        rden2 = ep.tile([128, 1], F32, tag="rden2")
                        nc.vector.reciprocal(out=rden2[:], in_=den2[:])
                        m2 = mp.tile([128, Lmax * Fout], BF, tag="m2")
                        p2e = p2[:].unsqueeze(2).to_broadcast([128, lt, Fout])
                        nc.vector.tensor_tensor(
                            out=m2[:, 0:lt * Fout].rearrange("p (l f) -> p l f", f=Fout),
                            in0=gv[:, :, 0:Fout], in1=p2e, op=mybir.AluOpType.mult)
                        ps2 = eps.tile([128, Fout], F32, tag="ps2")
                        for j in range(lt):
                            nc.tensor.matmul(ps2[:], lhsT=idb[:],
                                             rhs=m2[:, j * Fout:(j + 1) * Fout],
                                             start=(j == 0), stop=(j == lt - 1))
                        st["ps2"] = ps2
                        st["rden2"] = rden2
                    if 0 <= t - 3 < nblk:
                        b = t - 3
                        st = S[b]
                        ps2, rden2 = st["ps2"], st["rden2"]
                        o2 = ep.tile([128, Fout], F32, tag="o2")
                        r2e = rden2[:].to_broadcast([128, Fout])
                        nc.vector.tensor_tensor(out=o2[:], in0=ps2[:], in1=r2e,
                                                op=mybir.AluOpType.mult)
                        # o3 straight into the output buffer; softmax without
                        # max-shift (f32-safe range)
                        ob = obuf[:, b * Fout:(b + 1) * Fout]
                        nc.vector.tensor_tensor(out=ob, in0=o2[:], in1=b2c[:],
                                                op=mybir.AluOpType.add)
                        ex = ep.tile([128, Fout], F32, tag="ex")
                        nc.scalar.activation(out=ex[:], in_=ob,
                                             func=mybir.ActivationFunctionType.Exp,
                                             accum_out=sebuf[:, b:b + 1])
                        S[b] = {}
                # batched ln over all blocks, then one fused subtract + DMA out
                ls = cp.tile([128, nblk], F32)
                nc.scalar.activation(out=ls[:], in_=sebuf[:],
                                     func=mybir.ActivationFunctionType.Ln)
                ov = cp.tile([128, nblk * Fout], F32)
                lsv = ls[:].unsqueeze(2).to_broadcast([128, nblk, Fout])
                nc.vector.tensor_tensor(
                    out=ov[:].rearrange("p (b f) -> p b f", f=Fout),
                    in0=obuf[:].rearrange("p (b f) -> p b f", f=Fout),
                    in1=lsv, op=mybir.AluOpType.subtract)
                nc.sync.dma_start(out=outsh[:], in_=ov[:])
    nc.finalize()
    return nc


# ---------------------------------------------------------------- runner
_TRACE = False
last_times = {}


def _run_spmd(nc, in_maps, ncores):
    kw = {}
    if _TRACE:
        _install_hook()
        kw["trace"] = True
    return bass_utils.run_bass_kernel_spmd(nc, in_maps, core_ids=list(range(ncores)), **kw)


def _install_hook():
    try:
        import antenv
        if "antenv.axon_hooks" not in sys.modules:
            hooks_mod = types.ModuleType("antenv.axon_hooks")
            _h = [None]
            hooks_mod.set_axon_ntff_profile_hook = lambda h: _h.__setitem__(0, h)
            hooks_mod.get_axon_ntff_profile_hook = lambda: _h[0]
            sys.modules["antenv.axon_hooks"] = hooks_mod
            antenv.axon_hooks = hooks_mod
            from trn_agent_boot.trn_boot import _ntff_profile_via_ctypes
            hooks_mod.set_axon_ntff_profile_hook(
                _ntff_profile_via_ctypes('/opt/axon/libaxon_pjrt.so'))
    except Exception as e:  # pragma: no cover
        print("hook install failed:", e, file=sys.stderr)


def gat_forward(cfg, inputs):
    N, Fout, H, HC = cfg["N"], cfg["Fout"], cfg["H"], cfg["HC"]
    ncores, npc, nrows, nblk = cfg["ncores"], cfg["npc"], cfg["nrows"], cfg["nblk"]
    x = np.asarray(inputs["x"], np.float32)
    edge_index = np.asarray(inputs["edge_index"])

    g = preprocess_graph(cfg, edge_index)
    pp = preprocess_params(cfg, *[np.asarray(inputs[k]) for k in
                                  ("W1", "att_src1", "att_dst1", "b1", "bn_gamma",
                                   "bn_beta", "bn_mean", "bn_var", "W2",
                                   "att_src2", "att_dst2", "b2")])

    # ---- K1: per-core own-node transform
    nc1 = build_kernel_1(cfg)
    in1 = []
    for k in range(ncores):
        c = g["cores"][k]
        xtp = np.zeros((128, nrows), np.float32)
        valid = c["row2node"] >= 0
        xtp[:, valid] = x[c["row2node"][valid]].T
        in1.append({"xTP": xtp.astype(BF16), "W1cat2": pp["W1cat2"]})
    res1 = _run_spmd(nc1, in1, ncores)
    last_times["K1"] = res1.exec_time_ns

    # ---- host: assemble h|a_s table + per-core a_d, materialize slots
    htab_all = np.zeros((N + 1, KOUT), BF16)
    htab_all[N, HC:HC + H] = BF16(-1e30)           # sentinel: a_s = -inf
    for k in range(ncores):
        sh = res1.results[k]["htab"]
        c = g["cores"][k]
        valid = c["row2node"] >= 0
        htab_all[c["row2node"][valid]] = sh[valid]
    hg_cores = materialize_slots(cfg, g, htab_all[:, :W1CH], W1CH)

    nc2 = build_kernel_2(cfg, g)
    in2 = []
    for k in range(ncores):
        c = g["cores"][k]
        adall = np.zeros((128, nblk * H), BF16)
        r2n = c["row2node"].reshape(nblk, 128)
        for b in range(nblk):
            vb = r2n[b] >= 0
            adall[vb, b * H:(b + 1) * H] = htab_all[r2n[b][vb], HC + H:KOUT]
        in2.append({"hg": hg_cores[k], "adall": adall,
                    "b_bcast": pp["b_bcast"], "W2cat": pp["W2cat"],
                    "c2b": pp["c2b"], "identb": pp["identb"]})
    res2 = _run_spmd(nc2, in2, ncores)
    last_times["K2"] = res2.exec_time_ns

    # ---- host: assemble layer-2 table, materialize slots (f32)
    h2a_all = np.zeros((N + 1, W2CH), np.float32)
    h2a_all[N, Fout] = -1e30                       # sentinel: a_s2 = -inf
    for k in range(ncores):
        sh = res2.results[k]["shard"]
        c = g["cores"][k]
        valid = c["row2node"] >= 0
        h2a_all[c["row2node"][valid]] = sh[valid]
    hg2_cores = materialize_slots(cfg, g, h2a_all, W2CH)

    nc3 = build_kernel_3(cfg, g)
    in3 = []
    for k in range(ncores):
        c = g["cores"][k]
        ad2all = np.zeros((128, nblk), np.float32)
        r2n = c["row2node"].reshape(nblk, 128)
        for b in range(nblk):
            vb = r2n[b] >= 0
            ad2all[vb, b] = h2a_all[r2n[b][vb], Fout + 1]
        in3.append({"hg2": hg2_cores[k], "ad2all": ad2all,
                    "b2c": pp["b2c"], "identb": pp["identb"]})
    res3 = _run_spmd(nc3, in3, ncores)
    last_times["K3"] = res3.exec_time_ns

    out = np.zeros((N, Fout), np.float32)
    for k in range(ncores):
        sh = res3.results[k]["outsh"]              # [128, nblk*Fout]
        c = g["cores"][k]
        vals = sh.reshape(128, nblk, Fout).transpose(1, 0, 2).reshape(nrows, Fout)
        valid = c["row2node"] >= 0
        out[c["row2node"][valid]] = vals[valid]
    return out


def kernel(**inputs):
    cfg = make_cfg()
    return gat_forward(cfg, inputs)
